# revision 1
# baseline (speedup 1.0000x reference)
# Trainium2 Bass kernel for nn_CausalGCN (8-core SPMD).
#
# Sharding: nodes are split into 8 contiguous chunks (the batch vector is
# graph-sorted, so this is data-parallel over graphs). Each core owns the
# output rows of its chunk. Message passing uses the dma_gather /
# dma_scatter_add SWDGE instructions with int16 indices; gather tables are
# chunk PAIRS (2*12544 rows, int16-safe) assembled via AllGather collectives.
#
# Hardware-measured constraints honoured here:
#  - dma_scatter_add loses updates when one call carries duplicate target
#    rows (any partition/engine layout), so edges are split into occurrence
#    ROUNDS: each scatter call touches a destination row at most once.
#  - Sequential scatter calls to the same table serialize correctly.
#  - Pad positions all aim at a never-read dump row; races there are
#    harmless.
#  - Graph pooling has ~200-fold duplication, so it uses a one-hot matmul
#    instead of scatter-add.
#
# Self-contained: only numpy + concourse imports; no file I/O.
import numpy as np

F32np = np.float32

CFG_FULL = dict(N=100_000, E=1_000_000, F=128, H=64, NL=3, G=512, C=10, K=8,
                J=98, SPAN=80)

CALLCAP = 1024  # max num_idxs per dma_gather/dma_scatter_add call (HW limit)

# ---------------------------------------------------------------------------
# host-side preprocessing
# ---------------------------------------------------------------------------

def _wrap_idx16(arr):
    L = arr.shape[0]
    w = arr.reshape(L // 16, 16).T.astype(np.int16)
    return np.tile(w, (8, 1))


def _occurrence(key):
    """occ[i] = rank of i among equal key values (stable order)."""
    order = np.argsort(key, kind="stable")
    sk = key[order]
    n = len(sk)
    if not n:
        return np.zeros(0, np.int64)
    first = np.r_[0, np.nonzero(np.diff(sk))[0] + 1]
    starts = np.zeros(n, np.int64)
    starts[first] = first
    starts = np.maximum.accumulate(starts)
    occ = np.empty_like(order)
    occ[order] = np.arange(n) - starts
    return occ


def preprocess(edge_index, batch, cfg):
    N, K, G = cfg["N"], cfg["K"], cfg["G"]
    NC = N // K
    J = cfg["J"]
    NCP = 128 * J
    ZR = NC
    NPAIR = K // 2
    PAIRROWS = 2 * NCP
    assert PAIRROWS <= 32767
    SPAN = cfg["SPAN"]

    row = np.asarray(edge_index[0], dtype=np.int64)
    col = np.asarray(edge_index[1], dtype=np.int64)
    batch = np.asarray(batch, dtype=np.int64)

    cnt = np.bincount(row, minlength=N).astype(np.float64)
    dinv_st = (1.0 / np.sqrt(cnt + 1.0)).astype(F32np)
    norm_vals = (dinv_st[row] * dinv_st[col]).astype(F32np)

    ccol = col // NC
    crow = row // NC

    def pair_of(nodes):
        return (nodes // NC) // 2

    def pair_local(nodes):
        ch = nodes // NC
        return (ch % 2) * NCP + (nodes % NC)

    def make_stream(core_of, skey, gkey, with_norm):
        per_core = [np.nonzero(core_of == c)[0] for c in range(K)]
        occs = [_occurrence(skey[e]) for e in per_core]
        RMAX = max((int(o.max()) + 1 if o.size else 1) for o in occs)
        counts = np.zeros((K, RMAX, NPAIR), np.int64)
        for c in range(K):
            e = per_core[c]
            if e.size:
                np.add.at(counts[c], (occs[c], pair_of(gkey[e])), 1)
        seg = ((counts.max(axis=0) + 127) // 128) * 128
        base = np.full((RMAX, NPAIR), -1, np.int64)
        calls = []
        off = 0
        for r in range(RMAX):
            for p in range(NPAIR):
                n = int(seg[r, p])
                if n:
                    base[r, p] = off
                    for k0 in range(0, n, CALLCAP):
                        calls.append((off + k0, min(CALLCAP, n - k0), p))
                    off += n
        TOT = off
        g_arr = np.full((K, TOT), ZR, np.int16)
        s_arr = np.full((K, TOT), ZR, np.int16)
        n_arr = np.zeros((K, TOT), F32np) if with_norm else None
        for c in range(K):
            e = per_core[c]
            if not e.size:
                continue
            o = occs[c]
            p = pair_of(gkey[e])
            gl = pair_local(gkey[e])
            sl = skey[e] - c * NC
            order = np.lexsort((gl, p, o))
            segkey = o[order] * NPAIR + p[order]
            first = np.r_[0, np.nonzero(np.diff(segkey))[0] + 1]
            starts = np.zeros(len(e), np.int64)
            starts[first] = first
            starts = np.maximum.accumulate(starts)
            rank = np.arange(len(e)) - starts
            pos = np.empty(len(e), np.int64)
            pos[order] = base[o[order], p[order]] + rank
            g_arr[c, pos] = gl.astype(np.int16)
            s_arr[c, pos] = sl.astype(np.int16)
            if with_norm:
                n_arr[c, pos] = norm_vals[e]
        return calls, TOT, g_arr, s_arr, n_arr

    callsB, TOTB, gB, sB, nB = make_stream(ccol, col, row, True)
    callsR, TOTR, bR, aR, _ = make_stream(crow, row, col, False)

    def wrapK(a):
        K0, L = a.shape
        out = np.empty((K0, 128, L // 16), dtype=np.int16)
        for i in range(K0):
            out[i] = _wrap_idx16(a[i])
        return out

    nB_d = nB.reshape(K, TOTB // 128, 128).transpose(0, 2, 1).copy()

    def per_core_vec(full, pad=0.0):
        out = np.full((K, 128, J), pad, dtype=F32np)
        for c in range(K):
            v = np.full(NCP, pad, dtype=F32np)
            v[:NC] = full[c * NC:(c + 1) * NC]
            out[c] = v.reshape(128, J)
        return out

    dinv2_st = per_core_vec((dinv_st.astype(np.float64) ** 2).astype(F32np))
    cntp1 = per_core_vec((cnt + 1.0).astype(F32np), pad=1.0)
    mask = per_core_vec(np.ones(N, F32np))

    g0 = np.array([int(batch[c * NC]) for c in range(K)], np.int64)
    span_need = max(int(batch[(c + 1) * NC - 1]) - int(batch[c * NC]) + 1
                    for c in range(K))
    assert span_need <= SPAN, (span_need, SPAN)
    OH = np.zeros((K, J, 128, SPAN), F32np)
    for c in range(K):
        bl = (batch[c * NC:(c + 1) * NC] - g0[c]).astype(np.int64)
        n = np.arange(NC)
        OH[c, n % J, n // J, bl] = 1.0

    return dict(NC=NC, NCP=NCP, ZR=ZR, J=J, NPAIR=NPAIR, SPAN=SPAN,
                callsB=callsB, TOTB=TOTB, gB=wrapK(gB), sB=wrapK(sB),
                nB=nB_d,
                callsR=callsR, TOTR=TOTR, bR=wrapK(bR), aR=wrapK(aR),
                dinv2_st=dinv2_st, cntp1=cntp1, mask=mask,
                OH=OH, g0=[int(v) for v in g0])


def make_in_maps(inputs, cfg, pp):
    N, K, F, H, NL, G, C = (cfg["N"], cfg["K"], cfg["F"], cfg["H"], cfg["NL"],
                            cfg["G"], cfg["C"])
    NC, NCP, J = pp["NC"], pp["NCP"], pp["J"]
    f = lambda n: np.asarray(inputs[n], F32np)

    x = f("x")
    W_ea, b_ea = f("W_ea"), f("b_ea")
    W_na, b_na = f("W_na"), f("b_na")
    wa = (W_ea[:H, 0] - W_ea[:H, 1]).reshape(H, 1)
    wb = (W_ea[H:, 0] - W_ea[H:, 1]).reshape(H, 1)
    Wab = np.concatenate([wa, wb], axis=1).astype(F32np)
    deab = np.array([[b_ea[0] - b_ea[1]], [0.0]], dtype=F32np)
    wna = (W_na[:, 0] - W_na[:, 1]).reshape(H, 1).astype(F32np)
    dna = np.array([[b_na[0] - b_na[1]]], dtype=F32np)

    common = dict(
        Wfeat=f("W_feat"), Wconvs=f("W_convs"),
        bconvs=f("b_convs").reshape(NL, 1, H),
        Wab=Wab, deab=deab, wna=wna, dna=dna,
        Wctx=f("W_ctx"), bctx=f("b_ctx").reshape(1, H),
        Wobj=f("W_obj"), bobj=f("b_obj").reshape(1, H),
        W1_c=f("W1_c"), b1_c=f("b1_c").reshape(H, 1),
        W2_c=f("W2_c"), b2_c=f("b2_c").reshape(C, 1),
        W1_o=f("W1_o"), b1_o=f("b1_o").reshape(H, 1),
        W2_o=f("W2_o"), b2_o=f("b2_o").reshape(C, 1),
        W1_co=f("W1_co"), b1_co=f("b1_co").reshape(H, 1),
        W2_co=f("W2_co"), b2_co=f("b2_co").reshape(C, 1),
    )

    in_maps = []
    for c in range(K):
        xc = np.zeros((NCP, F), F32np)
        xc[:NC] = x[c * NC:(c + 1) * NC]
        m = dict(common)
        m["x_t"] = xc.reshape(128, J, F)
        m["dinv2_st"] = pp["dinv2_st"][c]
        m["cntp1"] = pp["cntp1"][c]
        m["mask"] = pp["mask"][c]
        m["gB"] = pp["gB"][c]
        m["sB"] = pp["sB"][c]
        m["nB"] = pp["nB"][c]
        m["aR"] = pp["aR"][c]
        m["bR"] = pp["bR"][c]
        m["OH"] = pp["OH"][c]
        in_maps.append(m)
    return in_maps


# ---------------------------------------------------------------------------
# device program
# ---------------------------------------------------------------------------

def build_program(cfg, meta):
    import concourse.bacc as bacc
    import concourse.mybir as mybir
    import concourse.tile as tile
    from concourse.masks import make_identity

    F32 = mybir.dt.float32
    I16 = mybir.dt.int16
    AF = mybir.ActivationFunctionType
    OP = mybir.AluOpType
    AX = mybir.AxisListType

    N, F, H, NL, G, C, K = (cfg["N"], cfg["F"], cfg["H"],
                            cfg["NL"], cfg["G"], cfg["C"], cfg["K"])
    NC, NCP, J = meta["NC"], meta["NCP"], meta["J"]
    NPAIR, SPAN = meta["NPAIR"], meta["SPAN"]
    callsB, TOTB = meta["callsB"], meta["TOTB"]
    callsR, TOTR = meta["callsR"], meta["TOTR"]
    g0 = meta["g0"]
    PAIRROWS = 2 * NCP
    MAXB = max(n for _, n, _ in callsB)
    MAXR = max(n for _, n, _ in callsR)
    RG = [list(range(K))]
    GJ = G // 128
    assert G % 128 == 0

    nc = bacc.Bacc("TRN2", target_bir_lowering=False, debug=False,
                   enable_asserts=False, num_devices=K)

    def din(name, shape, dt=F32):
        return nc.dram_tensor(name, list(shape), dt, kind="ExternalInput").ap()

    x_t = din("x_t", [128, J, F])
    Wfeat = din("Wfeat", [F, H])
    Wconvs = din("Wconvs", [NL, H, H])
    bconvs = din("bconvs", [NL, 1, H])
    Wab_i = din("Wab", [H, 2])
    deab_i = din("deab", [2, 1])
    wna_i = din("wna", [H, 1])
    dna_i = din("dna", [1, 1])
    Wctx_i = din("Wctx", [H, H])
    bctx_i = din("bctx", [1, H])
    Wobj_i = din("Wobj", [H, H])
    bobj_i = din("bobj", [1, H])
    rd_w = {}
    for t in ("c", "o", "co"):
        rd_w[t] = (din(f"W1_{t}", [H, H]), din(f"b1_{t}", [H, 1]),
                   din(f"W2_{t}", [H, C]), din(f"b2_{t}", [C, 1]))
    dinv2_i = din("dinv2_st", [128, J])
    cntp1_i = din("cntp1", [128, J])
    mask_i = din("mask", [128, J])
    gB_i = din("gB", [128, TOTB // 16], I16)
    sB_i = din("sB", [128, TOTB // 16], I16)
    nB_i = din("nB", [128, TOTB // 128])
    aR_i = din("aR", [128, TOTR // 16], I16)
    bR_i = din("bR", [128, TOTR // 16], I16)
    OH_i = din("OH", [J, 128, SPAN])

    out_t = nc.dram_tensor("out", [3, G, C], F32, kind="ExternalOutput").ap()

    def dram(name, shape, shared=False):
        return nc.dram_tensor(name, list(shape), F32, kind="Internal",
                              addr_space="Shared" if shared else "Local").ap()

    Mchunk = [dram(f"Mchunk{l}", [NCP, H]) for l in range(NL)]
    Mfull = [dram(f"Mfull{l}", [K * NCP, H], shared=True) for l in range(NL)]
    acc_l = [dram(f"acc{l}", [NCP, H]) for l in range(NL)]
    ab_in = dram("ab_in", [NCP, 2])
    ab_full = dram("ab_full", [K * NCP, 2], shared=True)
    Tpad = dram("Tpad", [K * NCP, H])
    Tpad_loc = dram("Tpad_loc", [NCP, H])
    degacc = dram("degacc", [NCP, H])
    MCOchunk = dram("MCOchunk", [NCP, 2 * H])
    MCOfull = dram("MCOfull", [K * NCP, 2 * H], shared=True)
    accCO = dram("accCO", [NCP, 2 * H])
    pool_part = dram("pool_part", [SPAN, 2 * H])
    pool_ag = dram("pool_ag", [K * SPAN, 2 * H], shared=True)
    pool_acc = dram("pool_acc", [G + SPAN, 2 * H])
    stat_in = {}
    stat_out = {}
    for nm, d, w in (("h0", F, 2), ("h1", H, 2), ("h2", H, 2), ("h3", H, 2),
                     ("co", H, 4)):
        stat_in[nm] = dram(f"stat_in_{nm}", [d, w])
        stat_out[nm] = dram(f"stat_out_{nm}", [d, w], shared=True)

    NCHUNK = -(-NCP // 512)

    def chunk_cols(k):
        c0 = k * 512
        return c0, min(512, NCP - c0)

    with tile.TileContext(nc) as tc:
        with tc.tile_pool(name="const", bufs=1) as CONST, \
             tc.tile_pool(name="work", bufs=1) as WORK, \
             tc.tile_pool(name="work2", bufs=2) as WORK2, \
             tc.tile_pool(name="ps", bufs=2, space="PSUM") as PS:

            ident = CONST.tile([128, 128], F32, tag="ident")
            make_identity(nc, ident[:])
            ones_row1 = CONST.tile([1, 128], F32, tag="ones_row1")
            nc.vector.memset(ones_row1[:], 1.0)
            ones_col = CONST.tile([128, 1], F32, tag="ones_col")
            nc.vector.memset(ones_col[:], 1.0)
            maskt = CONST.tile([128, J], F32, tag="maskt")
            nc.sync.dma_start(maskt[:], mask_i[:])
            dinv2t = CONST.tile([128, J], F32, tag="dinv2t")
            nc.sync.dma_start(dinv2t[:], dinv2_i[:])
            cntp1t = CONST.tile([128, J], F32, tag="cntp1t")
            nc.sync.dma_start(cntp1t[:], cntp1_i[:])
            dinv0 = CONST.tile([128, J], F32, tag="dinv0")
            dinv1 = CONST.tile([128, J], F32, tag="dinv1")
            wna_t = CONST.tile([H, 1], F32, tag="wna_t")
            nc.sync.dma_start(wna_t[:], wna_i[:])
            dna_t = CONST.tile([1, 1], F32, tag="dna_t")
            nc.sync.dma_start(dna_t[:], dna_i[:])

            zt = WORK.tile([128, 2048], F32, tag="ztile")
            nc.vector.memset(zt[:], 0.0)

            def zero_table(t, rows, d):
                flat = t[:].rearrange("(p j) d -> p (j d)", p=128)
                w = rows // 128 * d
                for c0 in range(0, w, 2048):
                    cw = min(2048, w - c0)
                    nc.sync.dma_start(flat[:, c0:c0 + cw], zt[:, :cw])

            for t in acc_l:
                zero_table(t, NCP, H)
            zero_table(degacc, NCP, H)
            zero_table(accCO, NCP, 2 * H)
            for r0 in range(0, G + SPAN, 128):
                rw = min(128, G + SPAN - r0)
                nc.sync.dma_start(pool_acc[r0:r0 + rw, :], zt[:rw, :2 * H])

            def bn_fold(stats2, Wi_dram, d_in, d_out, denom, extra_bias=None):
                Wi = WORK2.tile([d_in, d_out], F32, tag="Wi")
                nc.sync.dma_start(Wi[:], Wi_dram)
                ms = WORK2.tile([d_in, 2], F32, tag="ms")
                nc.vector.tensor_scalar_mul(ms[:], stats2[:], 1.0 / denom)
                var = WORK2.tile([d_in, 1], F32, tag="var")
                nc.vector.tensor_tensor(out=var[:], in0=ms[:, 0:1],
                                        in1=ms[:, 0:1], op=OP.mult)
                nc.vector.tensor_tensor(out=var[:], in0=ms[:, 1:2],
                                        in1=var[:], op=OP.subtract)
                nc.vector.tensor_scalar_add(var[:], var[:], 1e-5)
                sd = WORK2.tile([d_in, 1], F32, tag="sd")
                nc.scalar.activation(sd[:], var[:], AF.Sqrt)
                s = WORK2.tile([d_in, 1], F32, tag="s")
                nc.vector.reciprocal(s[:], sd[:])
                Wt = WORK2.tile([d_in, d_out], F32, tag="Wt")
                nc.vector.tensor_scalar_mul(Wt[:], Wi[:], s[:, 0:1])
                v = WORK2.tile([d_in, 1], F32, tag="v")
                nc.vector.tensor_tensor(out=v[:], in0=ms[:, 0:1], in1=s[:],
                                        op=OP.mult)
                nc.vector.tensor_scalar(out=v[:], in0=v[:], scalar1=-1.0,
                                        scalar2=1e-4, op0=OP.mult, op1=OP.add)
                pb = PS.tile([d_out, 1], F32, tag="paux", space="PSUM")
                nc.tensor.matmul(pb[:], lhsT=Wi[:], rhs=v[:], start=True,
                                 stop=True)
                bias = WORK2.tile([d_out, 1], F32, tag="bias")
                nc.scalar.activation(bias[:], pb[:], AF.Identity)
                if extra_bias is not None:
                    eb = WORK2.tile([d_out, 1], F32, tag="eb")
                    nc.sync.dma_start(eb[:], extra_bias)
                    nc.vector.tensor_tensor(out=bias[:], in0=bias[:],
                                            in1=eb[:], op=OP.add)
                return Wt, bias

            def allreduce_stats(nm, stats2):
                d, w = stats2.shape[0], stats2.shape[1]
                nc.sync.dma_start(stat_in[nm][:], stats2[:])
                nc.gpsimd.collective_compute(
                    "AllReduce", OP.add, replica_groups=RG,
                    ins=[stat_in[nm][:]], outs=[stat_out[nm][:]])
                back = WORK2.tile([d, w], F32, tag="stback")
                nc.sync.dma_start(back[:], stat_out[nm][:])
                return back

            def stats_nm(src_nm, d, sq_tile):
                part = WORK2.tile([128, d], F32, tag="statp")
                nc.vector.tensor_reduce(part[:],
                                        src_nm[:].rearrange("p j d -> p d j"),
                                        AX.X, OP.add)
                nc.scalar.activation(sq_tile[:], src_nm[:], AF.Square)
                part2 = WORK2.tile([128, d], F32, tag="statp2")
                nc.vector.tensor_reduce(part2[:],
                                        sq_tile[:].rearrange("p j d -> p d j"),
                                        AX.X, OP.add)
                stats2 = WORK2.tile([d, 2], F32, tag="st2")
                for i, p in enumerate((part, part2)):
                    pc = PS.tile([d, 1], F32, tag="paux", space="PSUM")
                    nc.tensor.matmul(pc[:], lhsT=p[:], rhs=ones_col[:],
                                     start=True, stop=True)
                    nc.scalar.activation(stats2[:, i:i + 1], pc[:],
                                         AF.Identity)
                return stats2

            def t_in_chunk(src_nm, k, d):
                c0, cw = chunk_cols(k)
                nj = cw // 128
                pt = PS.tile([d, 512], F32, tag="ptr", space="PSUM")
                for t in range(nj):
                    nc.tensor.transpose(pt[:, t * 128:(t + 1) * 128],
                                        src_nm[:, k * 4 + t, :], ident[:])
                hc = WORK2.tile([d, 512], F32, tag="hTc")
                nc.vector.tensor_copy(hc[:, :cw], pt[:, :cw])
                return hc, c0, cw, nj

            def t_out_chunk(mtc, k, d, dst_nm, dst_off, scale_nm):
                c0, cw = chunk_cols(k)
                nj = cw // 128
                pt = PS.tile([128, 4, 64], F32, tag="ptr", space="PSUM")
                for t in range(nj):
                    nc.tensor.transpose(pt[:, t, :d],
                                        mtc[:, t * 128:(t + 1) * 128],
                                        ident[:d, :d])
                j0 = k * 4
                nc.vector.tensor_tensor(
                    out=dst_nm[:, j0:j0 + nj, dst_off:dst_off + d],
                    in0=pt[:, :nj, :d],
                    in1=scale_nm[:, j0:j0 + nj, None].to_broadcast(
                        [128, nj, d]),
                    op=OP.mult)

            def narep_chunk(k, h_nm):
                hc, c0, cw, nj = t_in_chunk(h_nm, k, H)
                pn = PS.tile([1, 512], F32, tag="paux", space="PSUM")
                nc.tensor.matmul(pn[:, :cw], lhsT=wna_t[:], rhs=hc[:, :cw],
                                 start=True, stop=True)
                nrow = WORK2.tile([1, 512], F32, tag="narow")
                nc.scalar.activation(nrow[:, :cw], pn[:, :cw], AF.Sigmoid,
                                     bias=dna_t[:])
                pr = PS.tile([H, 512], F32, tag="paux", space="PSUM")
                nc.tensor.matmul(pr[:, :cw], lhsT=ones_row1[:, :H],
                                 rhs=nrow[:, :cw], start=True, stop=True)
                return hc, pr, c0, cw, nj

            with tc.tile_pool(name="bigc", bufs=1) as BIGC:
                hmA = BIGC.tile([128, J, H], F32, tag="hmA")
                hmB = BIGC.tile([128, J, H], F32, tag="hmB")

                # ---------- phase 1: x -> h0 (node-major) ----------
                with tc.tile_pool(name="px", bufs=1) as PX:
                    xnm = PX.tile([128, J, F], F32, tag="xnm")
                    nc.sync.dma_start(xnm[:], x_t[:])
                    part = WORK2.tile([128, F], F32, tag="statp")
                    nc.vector.tensor_reduce(
                        part[:], xnm[:].rearrange("p j d -> p d j"), AX.X,
                        OP.add)
                    part2 = WORK2.tile([128, F], F32, tag="statp2")
                    nc.vector.memset(part2[:], 0.0)
                    for k in range(NCHUNK):
                        c0, cw = chunk_cols(k)
                        nj = cw // 128
                        sqc = WORK2.tile([128, 4, F], F32, tag="sqc")
                        nc.scalar.activation(sqc[:, :nj, :],
                                             xnm[:, k * 4:k * 4 + nj, :],
                                             AF.Square)
                        ptmp = WORK2.tile([128, F], F32, tag="ptmp")
                        nc.vector.tensor_reduce(
                            ptmp[:],
                            sqc[:, :nj, :].rearrange("p j d -> p d j"),
                            AX.X, OP.add)
                        nc.vector.tensor_tensor(out=part2[:], in0=part2[:],
                                                in1=ptmp[:], op=OP.add)
                    st0 = WORK2.tile([F, 2], F32, tag="st2")
                    for i, p in enumerate((part, part2)):
                        pc = PS.tile([F, 1], F32, tag="paux", space="PSUM")
                        nc.tensor.matmul(pc[:], lhsT=p[:], rhs=ones_col[:],
                                         start=True, stop=True)
                        nc.scalar.activation(st0[:, i:i + 1], pc[:],
                                             AF.Identity)
                    st0 = allreduce_stats("h0", st0)
                    Wt0, bias0 = bn_fold(st0, Wfeat[:], F, H, float(N))
                    for k in range(NCHUNK):
                        hc, c0, cw, nj = t_in_chunk(xnm, k, F)
                        pm = PS.tile([H, 512], F32, tag="pmm", space="PSUM")
                        nc.tensor.matmul(pm[:, :cw], lhsT=Wt0[:],
                                         rhs=hc[:, :cw], start=True,
                                         stop=True)
                        mtc = WORK2.tile([H, 512], F32, tag="mtc")
                        nc.scalar.activation(mtc[:, :cw], pm[:, :cw], AF.Relu,
                                             bias=bias0[:])
                        t_out_chunk(mtc, k, H, hmA, 0, maskt)

                # ---------- GCN layers ----------
                h_nm = hmA
                other = hmB
                for l in range(NL):
                    sqs = BIGC.tile([128, J, H], F32, tag="sqs")
                    st = stats_nm(h_nm, H, sqs)
                    st = allreduce_stats(f"h{l + 1}" if l < 2 else "h3", st)
                    Wt_l, bias_l = bn_fold(st, Wconvs[l, :, :], H, H, float(N))

                    Mtile = other
                    for k in range(NCHUNK):
                        hc, c0, cw, nj = t_in_chunk(h_nm, k, H)
                        pm = PS.tile([H, 512], F32, tag="pmm", space="PSUM")
                        nc.tensor.matmul(pm[:, :cw], lhsT=Wt_l[:],
                                         rhs=hc[:, :cw], start=True,
                                         stop=True)
                        mtc = WORK2.tile([H, 512], F32, tag="mtc")
                        nc.scalar.activation(mtc[:, :cw], pm[:, :cw],
                                             AF.Identity, bias=bias_l[:])
                        t_out_chunk(mtc, k, H, Mtile, 0, maskt)
                    nc.sync.dma_start(
                        Mchunk[l][:].rearrange("(p j) d -> p j d", p=128),
                        Mtile[:])
                    nc.gpsimd.collective_compute(
                        "AllGather", OP.bypass, replica_groups=RG,
                        ins=[Mchunk[l][:]], outs=[Mfull[l][:]])

                    with tc.tile_pool(name="sl", bufs=1) as SLI, \
                         tc.tile_pool(name="slm", bufs=3) as SLM:
                        gbt = SLI.tile([128, TOTB // 16], I16, tag="gbt")
                        nc.sync.dma_start(gbt[:], gB_i[:])
                        sbt = SLI.tile([128, TOTB // 16], I16, tag="sbt")
                        nc.sync.dma_start(sbt[:], sB_i[:])
                        nrm = SLI.tile([128, TOTB // 128], F32, tag="nrm")
                        nc.sync.dma_start(nrm[:], nB_i[:])
                        for (off, n, p) in callsB:
                            nb = n // 128
                            msg = SLM.tile([128, MAXB // 128, H], F32,
                                           tag="msgL")
                            nc.gpsimd.dma_gather(
                                out_ap=msg[:, :nb, :],
                                in_ap=Mfull[l][p * PAIRROWS:
                                               (p + 1) * PAIRROWS, :],
                                idxs_ap=gbt[:, off // 16:(off + n) // 16],
                                num_idxs=n, num_idxs_reg=n, elem_size=H)
                            nc.vector.tensor_tensor(
                                out=msg[:, :nb, :], in0=msg[:, :nb, :],
                                in1=nrm[:, off // 128:off // 128 + nb, None
                                        ].to_broadcast([128, nb, H]),
                                op=OP.mult)
                            nc.gpsimd.dma_scatter_add(
                                acc_l[l][:], msg[:, :nb, :],
                                sbt[:, off // 16:(off + n) // 16], n, n, H)

                    atile = sqs
                    nc.sync.dma_start(
                        atile[:],
                        acc_l[l][:].rearrange("(p j) d -> p j d", p=128))
                    brow = WORK2.tile([1, H], F32, tag="brow")
                    nc.sync.dma_start(brow[:], bconvs[l, :, :])
                    brep = WORK2.tile([128, H], F32, tag="brep")
                    nc.gpsimd.partition_broadcast(brep[:], brow[:])
                    hn = Mtile
                    nc.vector.tensor_tensor(
                        out=hn[:], in0=Mtile[:],
                        in1=dinv2t[:, :, None].to_broadcast([128, J, H]),
                        op=OP.mult)
                    nc.vector.tensor_tensor(out=hn[:], in0=hn[:],
                                            in1=atile[:], op=OP.add)
                    nc.vector.tensor_tensor(
                        out=hn[:], in0=hn[:],
                        in1=brep[:, None, :].to_broadcast([128, J, H]),
                        op=OP.add)
                    nc.scalar.activation(hn[:], hn[:], AF.Relu)
                    nc.vector.tensor_tensor(
                        out=hn[:], in0=hn[:],
                        in1=maskt[:, :, None].to_broadcast([128, J, H]),
                        op=OP.mult)
                    h_nm, other = hn, h_nm

                h3 = h_nm

                # ---------- ab projections + xc/xo stats ----------
                Wab_t = WORK2.tile([H, 2], F32, tag="Wab_t")
                nc.sync.dma_start(Wab_t[:], Wab_i[:])
                deab_t = WORK2.tile([2, 1], F32, tag="deab_t")
                nc.sync.dma_start(deab_t[:], deab_i[:])
                abnm = WORK.tile([128, J, 2], F32, tag="abnm")
                stco = WORK.tile([H, 4], F32, tag="stco")
                sc25 = WORK.tile([H, NCHUNK, 4], F32, tag="sc25")
                for k in range(NCHUNK):
                    hc, pr, c0, cw, nj = narep_chunk(k, h3)
                    pm = PS.tile([2, 512], F32, tag="pmm", space="PSUM")
                    nc.tensor.matmul(pm[:, :cw], lhsT=Wab_t[:], rhs=hc[:, :cw],
                                     start=True, stop=True)
                    abc = WORK2.tile([2, 512], F32, tag="abc")
                    nc.scalar.activation(abc[:, :cw], pm[:, :cw],
                                         AF.Identity, bias=deab_t[:])
                    t_out_chunk(abc, k, 2, abnm, 0, maskt)
                    xck = WORK2.tile([H, 512], F32, tag="xck")
                    nc.vector.tensor_tensor(out=xck[:, :cw], in0=hc[:, :cw],
                                            in1=pr[:, :cw], op=OP.mult)
                    xok = WORK2.tile([H, 512], F32, tag="xok")
                    nc.vector.tensor_tensor(out=xok[:, :cw], in0=hc[:, :cw],
                                            in1=xck[:, :cw], op=OP.subtract)
                    nc.vector.tensor_reduce(sc25[:, k, 0:1], xck[:, :cw],
                                            AX.X, OP.add)
                    nc.vector.tensor_reduce(sc25[:, k, 2:3], xok[:, :cw],
                                            AX.X, OP.add)
                    psq = PS.tile([H, 512], F32, tag="paux", space="PSUM")
                    nc.scalar.activation(psq[:, :cw], xck[:, :cw], AF.Square,
                                         accum_out=sc25[:, k, 1:2])
                    psq2 = PS.tile([H, 512], F32, tag="paux", space="PSUM")
                    nc.scalar.activation(psq2[:, :cw], xok[:, :cw], AF.Square,
                                         accum_out=sc25[:, k, 3:4])
                for q in range(4):
                    nc.vector.tensor_reduce(stco[:, q:q + 1], sc25[:, :, q],
                                            AX.X, OP.add)
                nc.sync.dma_start(
                    ab_in[:].rearrange("(p j) d -> p j d", p=128), abnm[:])
                nc.gpsimd.collective_compute(
                    "AllGather", OP.bypass, replica_groups=RG,
                    ins=[ab_in[:]], outs=[ab_full[:]])
                stco_b = allreduce_stats("co", stco)

                # ---------- Tpad tables ----------
                with tc.tile_pool(name="tp", bufs=2) as TP:
                    def build_tpad(dst, src2):
                        ab2 = TP.tile([128, J, 2], F32, tag="tp_ab")
                        nc.sync.dma_start(
                            ab2[:], src2.rearrange("(p j) d -> p j d", p=128))
                        tt = TP.tile([128, J, H], F32, tag="tp_t")
                        nc.vector.memset(tt[:], 0.0)
                        nc.vector.tensor_copy(tt[:, :, 0:2], ab2[:])
                        nc.sync.dma_start(
                            dst.rearrange("(p j) d -> p j d", p=128), tt[:])

                    for s in range(K):
                        build_tpad(Tpad[s * NCP:(s + 1) * NCP, :],
                                   ab_full[s * NCP:(s + 1) * NCP, :])
                    build_tpad(Tpad_loc[:], ab_in[:])

                # ---------- stream R: dynamic degree ----------
                with tc.tile_pool(name="sri", bufs=1) as SRI, \
                     tc.tile_pool(name="srm", bufs=2) as SRM:
                    dmsg = []
                    for i in range(2):
                        d = SRI.tile([128, MAXR // 128, H], F32,
                                     tag=f"dmsg{i}")
                        nc.vector.memset(d[:], 0.0)
                        dmsg.append(d)
                    bi = 0
                    for (off, n, p) in callsR:
                        nb = n // 128
                        ark = SRM.tile([128, MAXR // 16], I16, tag="ark")
                        nc.sync.dma_start(ark[:, :n // 16],
                                          aR_i[:, off // 16:(off + n) // 16])
                        brk = SRM.tile([128, MAXR // 16], I16, tag="brk")
                        nc.sync.dma_start(brk[:, :n // 16],
                                          bR_i[:, off // 16:(off + n) // 16])
                        at = SRM.tile([128, MAXR // 128, H], F32, tag="atR")
                        nc.gpsimd.dma_gather(
                            out_ap=at[:, :nb, :], in_ap=Tpad_loc[:],
                            idxs_ap=ark[:, :n // 16],
                            num_idxs=n, num_idxs_reg=n, elem_size=H)
                        bt = SRM.tile([128, MAXR // 128, H], F32, tag="btR")
                        nc.gpsimd.dma_gather(
                            out_ap=bt[:, :nb, :],
                            in_ap=Tpad[p * PAIRROWS:(p + 1) * PAIRROWS, :],
                            idxs_ap=brk[:, :n // 16],
                            num_idxs=n, num_idxs_reg=n, elem_size=H)
                        d = dmsg[bi % 2]
                        bi += 1
                        nc.vector.tensor_tensor(out=d[:, :nb, 0:1],
                                                in0=at[:, :nb, 0:1],
                                                in1=bt[:, :nb, 1:2],
                                                op=OP.add)
                        nc.scalar.activation(d[:, :nb, 0:1], d[:, :nb, 0:1],
                                             AF.Sigmoid)
                        nc.gpsimd.dma_scatter_add(
                            degacc[:], d[:, :nb, :],
                            ark[:, :n // 16], n, n, H)

                dtile = BIGC.tile([128, J, H], F32, tag="sqs")
                nc.sync.dma_start(
                    dtile[:], degacc[:].rearrange("(p j) d -> p j d", p=128))
                S0 = WORK2.tile([128, J], F32, tag="S0")
                nc.vector.tensor_copy(S0[:], dtile[:, :, 0])
                sd0 = WORK2.tile([128, J], F32, tag="sd0")
                nc.scalar.activation(sd0[:], S0[:], AF.Sqrt, bias=1.0)
                nc.vector.reciprocal(dinv0[:], sd0[:])
                nc.vector.tensor_tensor(out=dinv0[:], in0=dinv0[:],
                                        in1=maskt[:], op=OP.mult)
                dg1 = WORK2.tile([128, J], F32, tag="dg1")
                nc.vector.tensor_tensor(out=dg1[:], in0=cntp1t[:], in1=S0[:],
                                        op=OP.subtract)
                nc.vector.tensor_scalar_max(dg1[:], dg1[:], 1e-20)
                nc.scalar.activation(dg1[:], dg1[:], AF.Sqrt)
                nc.vector.reciprocal(dinv1[:], dg1[:])
                nc.vector.tensor_tensor(out=dinv1[:], in0=dinv1[:],
                                        in1=maskt[:], op=OP.mult)

                # ---------- Mc'/Mo' -> MCO ----------
                Wt_c, bias_c = bn_fold(stco_b[:, 0:2], Wctx_i[:], H, H,
                                       float(N))
                WtC = WORK.tile([H, H], F32, tag="WtC")
                nc.vector.tensor_copy(WtC[:], Wt_c[:])
                bC = WORK.tile([H, 1], F32, tag="bC")
                nc.vector.tensor_copy(bC[:], bias_c[:])
                Wt_o, bias_o = bn_fold(stco_b[:, 2:4], Wobj_i[:], H, H,
                                       float(N))

                with tc.tile_pool(name="pmco", bufs=1) as PMCO:
                    MCO = PMCO.tile([128, J, 2 * H], F32, tag="MCO")
                    for k in range(NCHUNK):
                        hc, pr, c0, cw, nj = narep_chunk(k, h3)
                        xck = WORK2.tile([H, 512], F32, tag="xck")
                        nc.vector.tensor_tensor(out=xck[:, :cw],
                                                in0=hc[:, :cw],
                                                in1=pr[:, :cw], op=OP.mult)
                        xok = WORK2.tile([H, 512], F32, tag="xok")
                        nc.vector.tensor_tensor(out=xok[:, :cw],
                                                in0=hc[:, :cw],
                                                in1=xck[:, :cw],
                                                op=OP.subtract)
                        for half, (xk, Wt_h, bias_h, dinv_h) in enumerate(
                                ((xck, WtC, bC, dinv0),
                                 (xok, Wt_o, bias_o, dinv1))):
                            pm = PS.tile([H, 512], F32, tag="pmm",
                                         space="PSUM")
                            nc.tensor.matmul(pm[:, :cw], lhsT=Wt_h[:],
                                             rhs=xk[:, :cw], start=True,
                                             stop=True)
                            mtc = WORK2.tile([H, 512], F32, tag="mtc")
                            nc.scalar.activation(mtc[:, :cw], pm[:, :cw],
                                                 AF.Identity, bias=bias_h[:])
                            t_out_chunk(mtc, k, H, MCO, half * H, dinv_h)
                    nc.sync.dma_start(
                        MCOchunk[:].rearrange("(p j) d -> p j d", p=128),
                        MCO[:])
                nc.gpsimd.collective_compute(
                    "AllGather", OP.bypass, replica_groups=RG,
                    ins=[MCOchunk[:]], outs=[MCOfull[:]])

            # ---------- ctx/obj message stream ----------
            with tc.tile_pool(name="sci", bufs=1) as SCI, \
                 tc.tile_pool(name="scm", bufs=2) as SCM:
                gbt = SCI.tile([128, TOTB // 16], I16, tag="gbt2")
                nc.sync.dma_start(gbt[:], gB_i[:])
                sbt = SCI.tile([128, TOTB // 16], I16, tag="sbt2")
                nc.sync.dma_start(sbt[:], sB_i[:])
                for (off, n, p) in callsB:
                    nb = n // 128
                    i0 = off // 16
                    i1 = (off + n) // 16
                    mt = SCM.tile([128, MAXB // 128, 2 * H], F32, tag="mtC")
                    nc.gpsimd.dma_gather(
                        out_ap=mt[:, :nb, :],
                        in_ap=MCOfull[p * PAIRROWS:(p + 1) * PAIRROWS, :],
                        idxs_ap=gbt[:, i0:i1],
                        num_idxs=n, num_idxs_reg=n, elem_size=2 * H)
                    at = SCM.tile([128, MAXB // 128, H], F32, tag="atC")
                    nc.gpsimd.dma_gather(
                        out_ap=at[:, :nb, :],
                        in_ap=Tpad[p * PAIRROWS:(p + 1) * PAIRROWS, :],
                        idxs_ap=gbt[:, i0:i1],
                        num_idxs=n, num_idxs_reg=n, elem_size=H)
                    bt = SCM.tile([128, MAXB // 128, H], F32, tag="btC")
                    nc.gpsimd.dma_gather(
                        out_ap=bt[:, :nb, :], in_ap=Tpad_loc[:],
                        idxs_ap=sbt[:, i0:i1],
                        num_idxs=n, num_idxs_reg=n, elem_size=H)
                    ea = SCM.tile([128, MAXB // 128, 1], F32, tag="eaC")
                    nc.vector.tensor_tensor(out=ea[:, :nb, :],
                                            in0=at[:, :nb, 0:1],
                                            in1=bt[:, :nb, 1:2], op=OP.add)
                    nc.scalar.activation(ea[:, :nb, :], ea[:, :nb, :],
                                         AF.Sigmoid)
                    ea1 = SCM.tile([128, MAXB // 128, 1], F32, tag="ea1C")
                    nc.vector.tensor_scalar(out=ea1[:, :nb, :],
                                            in0=ea[:, :nb, :], scalar1=-1.0,
                                            scalar2=1.0, op0=OP.mult,
                                            op1=OP.add)
                    nc.vector.tensor_tensor(
                        out=mt[:, :nb, 0:H], in0=mt[:, :nb, 0:H],
                        in1=ea[:, :nb, :].to_broadcast([128, nb, H]),
                        op=OP.mult)
                    nc.vector.tensor_tensor(
                        out=mt[:, :nb, H:2 * H], in0=mt[:, :nb, H:2 * H],
                        in1=ea1[:, :nb, :].to_broadcast([128, nb, H]),
                        op=OP.mult)
                    nc.gpsimd.dma_scatter_add(
                        accCO[:], mt[:, :nb, :], sbt[:, i0:i1], n, n, 2 * H)

            # ---------- xc_f / xo_f, pooling, readout ----------
            with tc.tile_pool(name="bp", bufs=1) as BP:
                at2 = BP.tile([128, J, 2 * H], F32, tag="at2")
                nc.sync.dma_start(
                    at2[:], accCO[:].rearrange("(p j) d -> p j d", p=128))
                for half, dinv_h in ((0, dinv0), (1, dinv1)):
                    mch = BP.tile([128, J, H], F32, tag="mch")
                    nc.sync.dma_start(
                        mch[:],
                        MCOchunk[:, half * H:(half + 1) * H].rearrange(
                            "(p j) d -> p j d", p=128))
                    nc.vector.tensor_tensor(
                        out=at2[:, :, half * H:(half + 1) * H],
                        in0=at2[:, :, half * H:(half + 1) * H],
                        in1=mch[:], op=OP.add)
                    nc.vector.tensor_tensor(
                        out=at2[:, :, half * H:(half + 1) * H],
                        in0=at2[:, :, half * H:(half + 1) * H],
                        in1=dinv_h[:, :, None].to_broadcast([128, J, H]),
                        op=OP.mult)
                bco_row = WORK2.tile([1, 2 * H], F32, tag="bco_row")
                nc.sync.dma_start(bco_row[:, 0:H], bctx_i[:])
                nc.sync.dma_start(bco_row[:, H:2 * H], bobj_i[:])
                bco_rep = WORK.tile([128, 2 * H], F32, tag="bco_rep")
                nc.gpsimd.partition_broadcast(bco_rep[:], bco_row[:])
                nc.vector.tensor_tensor(
                    out=at2[:], in0=at2[:],
                    in1=bco_rep[:, None, :].to_broadcast([128, J, 2 * H]),
                    op=OP.add)
                nc.scalar.activation(at2[:], at2[:], AF.Relu)
                nc.vector.tensor_tensor(
                    out=at2[:], in0=at2[:],
                    in1=maskt[:, :, None].to_broadcast([128, J, 2 * H]),
                    op=OP.mult)

                # pooling via one-hot matmul over node blocks
                OHt = BP.tile([128, J, SPAN], F32, tag="OHt")
                nc.sync.dma_start(OHt[:], OH_i[:].rearrange("j p q -> p j q"))
                ppool = PS.tile([SPAN, 2 * H], F32, tag="pmm", space="PSUM")
                for j in range(J):
                    nc.tensor.matmul(ppool[:], lhsT=OHt[:, j, :],
                                     rhs=at2[:, j, :], start=(j == 0),
                                     stop=(j == J - 1))
                ppart = WORK.tile([SPAN, 2 * H], F32, tag="ppart")
                nc.scalar.activation(ppart[:], ppool[:], AF.Identity)
                nc.sync.dma_start(pool_part[:], ppart[:])
                nc.gpsimd.collective_compute(
                    "AllGather", OP.bypass, replica_groups=RG,
                    ins=[pool_part[:]], outs=[pool_ag[:]])
                for c2 in range(K):
                    seg = WORK2.tile([SPAN, 2 * H], F32, tag="pseg")
                    nc.sync.dma_start(seg[:],
                                      pool_ag[c2 * SPAN:(c2 + 1) * SPAN, :])
                    cur = WORK2.tile([SPAN, 2 * H], F32, tag="pcur")
                    nc.sync.dma_start(cur[:],
                                      pool_acc[g0[c2]:g0[c2] + SPAN, :])
                    nc.vector.tensor_tensor(out=cur[:], in0=cur[:],
                                            in1=seg[:], op=OP.add)
                    nc.sync.dma_start(pool_acc[g0[c2]:g0[c2] + SPAN, :],
                                      cur[:])

                gt = BP.tile([128, GJ, 2 * H], F32, tag="gt")
                nc.sync.dma_start(
                    gt[:],
                    pool_acc[0:G, :].rearrange("(p j) d -> p j d", p=128))
                gcT = WORK.tile([H, G], F32, tag="gcT")
                goT = WORK.tile([H, G], F32, tag="goT")
                gcoT = WORK.tile([H, G], F32, tag="gcoT")
                for half, dst in ((0, gcT), (1, goT)):
                    pt = PS.tile([H, 512], F32, tag="ptr", space="PSUM")
                    for t in range(GJ):
                        nc.tensor.transpose(pt[:, t * 128:(t + 1) * 128],
                                            gt[:, t, half * H:(half + 1) * H],
                                            ident[:])
                    nc.vector.tensor_copy(dst[:], pt[:, :G])
                nc.vector.tensor_tensor(out=gcoT[:], in0=gcT[:], in1=goT[:],
                                        op=OP.add)

                def readout(zT, wkey, out_idx):
                    W1, b1, W2, b2 = rd_w[wkey]
                    st2 = WORK2.tile([H, 2], F32, tag="st2")
                    nc.vector.tensor_reduce(st2[:, 0:1], zT[:], AX.X, OP.add)
                    psq = PS.tile([H, 512], F32, tag="paux", space="PSUM")
                    nc.scalar.activation(psq[:, :G], zT[:], AF.Square,
                                         accum_out=st2[:, 1:2])
                    Wt1, bias1 = bn_fold(st2, W1[:], H, H, float(G),
                                         extra_bias=b1[:])
                    pm = PS.tile([H, 512], F32, tag="pmm", space="PSUM")
                    nc.tensor.matmul(pm[:, :G], lhsT=Wt1[:], rhs=zT[:],
                                     start=True, stop=True)
                    z1T = WORK2.tile([H, G], F32, tag="rd_z1")
                    nc.scalar.activation(z1T[:], pm[:, :G], AF.Relu,
                                         bias=bias1[:])
                    st2b = WORK2.tile([H, 2], F32, tag="st2b")
                    nc.vector.tensor_reduce(st2b[:, 0:1], z1T[:], AX.X,
                                            OP.add)
                    psq2 = PS.tile([H, 512], F32, tag="paux", space="PSUM")
                    nc.scalar.activation(psq2[:, :G], z1T[:], AF.Square,
                                         accum_out=st2b[:, 1:2])
                    Wt2, bias2 = bn_fold(st2b, W2[:], H, C, float(G),
                                         extra_bias=b2[:])
                    pm2 = PS.tile([C, 512], F32, tag="paux", space="PSUM")
                    nc.tensor.matmul(pm2[:, :G], lhsT=Wt2[:], rhs=z1T[:],
                                     start=True, stop=True)
                    z2T = WORK2.tile([C, G], F32, tag="rd_z2")
                    nc.scalar.activation(z2T[:], pm2[:, :G], AF.Identity,
                                         bias=bias2[:])
                    z2 = WORK2.tile([128, GJ, C], F32, tag="rd_z2nm")
                    pt = PS.tile([128, GJ, C], F32, tag="ptr", space="PSUM")
                    for t in range(GJ):
                        nc.tensor.transpose(pt[:, t, :C],
                                            z2T[:, t * 128:(t + 1) * 128],
                                            ident[:C, :C])
                    nc.vector.tensor_copy(z2[:], pt[:])
                    mx = WORK2.tile([128, GJ], F32, tag="rd_mx")
                    nc.vector.tensor_reduce(mx[:], z2[:], AX.X, OP.max)
                    nc.vector.tensor_tensor(
                        out=z2[:], in0=z2[:],
                        in1=mx[:, :, None].to_broadcast([128, GJ, C]),
                        op=OP.subtract)
                    ex = WORK2.tile([128, GJ, C], F32, tag="rd_ex")
                    nc.scalar.activation(ex[:], z2[:], AF.Exp)
                    se = WORK2.tile([128, GJ], F32, tag="rd_se")
                    nc.vector.tensor_reduce(se[:], ex[:], AX.X, OP.add)
                    nc.scalar.activation(se[:], se[:], AF.Ln)
                    nc.vector.tensor_tensor(
                        out=z2[:], in0=z2[:],
                        in1=se[:, :, None].to_broadcast([128, GJ, C]),
                        op=OP.subtract)
                    nc.sync.dma_start(
                        out_t[out_idx, :, :].rearrange("(p j) c -> p j c",
                                                       p=128),
                        z2[:])

                readout(gcT[:], "c", 0)
                readout(goT[:], "o", 1)
                readout(gcoT[:], "co", 2)

    return nc


# ---------------------------------------------------------------------------
# entry point
# ---------------------------------------------------------------------------

def kernel(**inputs):
    cfg = dict(CFG_FULL)
    pp = preprocess(np.asarray(inputs["edge_index"]),
                    np.asarray(inputs["batch"]), cfg)
    in_maps = make_in_maps(inputs, cfg, pp)
    nc = build_program(cfg, pp)
    nc.compile()
    from concourse.bass_utils import run_bass_kernel_spmd
    res = run_bass_kernel_spmd(nc, in_maps, core_ids=list(range(cfg["K"])))
    return np.asarray(res.results[0]["out"])

